# revision 1
# baseline (speedup 1.0000x reference)
"""ALiBi bias kernel for Trainium2, SPMD across 8 NeuronCores.

out[b, h, i, j] = scores[b, h, i, j] - slope[h] * (i - j)

(The `offset` input cancels: (i+off) - (j+off) == i - j exactly in f32 for
integer offsets well inside the f32 exact-integer range.)

Sharding: flatten [B, H] = [2, 16] -> 32 slices; each of the 8 cores owns 4
consecutive (b, h) slices. The bias only depends on (h, i - j), so each core
receives a host-precomputed "strip" per slice: strip[p, c] = slope_h * (p - c
+ 1920) of shape [128, 3968]. For the row-tile starting at row r0, the bias
tile [128, 2048] is exactly strip[:, 1920-r0 : 1920-r0+2048], so the whole
kernel is one tensor_sub per tile.
"""

import numpy as np

_B, _H, _S = 2, 16, 2048
_NC = 8
_SPC = (_B * _H) // _NC  # slices (b,h pairs) per core = 4
_P = 128                 # SBUF partitions / row-tile height
_PAD = _S - _P           # 1920
_SW = _S + _PAD          # strip width 3968
_NRT = _S // _P          # row tiles per slice = 16

_CACHE = {}


def _build_nc():
    import concourse.tile as tile
    from concourse import bacc, mybir

    f32 = mybir.dt.float32
    nc = bacc.Bacc("TRN2", target_bir_lowering=False, debug=False)
    scores_in = nc.declare_dram_parameter("scores", [_SPC, _S, _S], f32, isOutput=False)
    strips_in = nc.declare_dram_parameter("strips", [_P, _SPC * _SW], f32, isOutput=False)
    out_ext = nc.declare_dram_parameter("out", [_SPC, _S, _S], f32, isOutput=True)

    with tile.TileContext(nc) as tc:
        with (
            tc.tile_pool(name="strip", bufs=1) as sp,
            tc.tile_pool(name="inp", bufs=4) as ip,
            tc.tile_pool(name="outp", bufs=4) as op,
        ):
            strips = sp.tile([_P, _SPC * _SW], f32)
            nc.sync.dma_start(strips[:], strips_in[:])
            for hl in range(_SPC):
                for r in range(_NRT):
                    r0 = r * _P
                    t = ip.tile([_P, _S], f32)
                    nc.sync.dma_start(t[:], scores_in[hl, r0 : r0 + _P, :])
                    o = op.tile([_P, _S], f32)
                    off = hl * _SW + (_PAD - r0)
                    nc.vector.tensor_sub(o[:], t[:], strips[:, off : off + _S])
                    nc.sync.dma_start(out_ext[hl, r0 : r0 + _P, :], o[:])
    nc.compile()
    return nc


def _strips_np():
    # slopes as the reference computes them (f32 throughout)
    slopes = (
        2.0 ** (-8.0 * np.arange(1, _H + 1, dtype=np.float32) / np.float32(_H))
    ).astype(np.float32)
    p = np.arange(_P, dtype=np.float32)[:, None]
    c = np.arange(_SW, dtype=np.float32)[None, :]
    base = (p - c) + np.float32(_PAD)  # exact small integers in f32
    strips = np.empty((_NC, _P, _SPC * _SW), dtype=np.float32)
    for core in range(_NC):
        for hl in range(_SPC):
            h = (core * _SPC + hl) % _H
            strips[core, :, hl * _SW : (hl + 1) * _SW] = slopes[h] * base
    return strips


def run(scores, offset=0, trace=False, trace_kwargs=None):
    """Run the SPMD kernel; returns (full_output, BassKernelResults)."""
    from concourse.bass_utils import run_bass_kernel_spmd

    scores = np.asarray(scores)
    assert scores.shape == (_B, _H, _S, _S) and scores.dtype == np.float32

    if "nc" not in _CACHE:
        _CACHE["nc"] = _build_nc()
        _CACHE["strips"] = _strips_np()
    nc = _CACHE["nc"]
    strips = _CACHE["strips"]

    flat = scores.reshape(_B * _H, _S, _S)
    in_maps = [
        {"scores": flat[c * _SPC : (c + 1) * _SPC], "strips": strips[c]}
        for c in range(_NC)
    ]
    res = run_bass_kernel_spmd(
        nc,
        in_maps,
        core_ids=list(range(_NC)),
        trace=trace,
        **(trace_kwargs or {}),
    )
    out = np.empty((_B * _H, _S, _S), dtype=np.float32)
    for c in range(_NC):
        out[c * _SPC : (c + 1) * _SPC] = res.results[c]["out"]
    return out.reshape(_B, _H, _S, _S), res


def kernel(scores, offset=0):
    out, _ = run(scores, offset=offset, trace=False)
    return out


# revision 4
# speedup vs baseline: 1.0363x; 1.0363x over previous
"""ALiBi bias kernel for Trainium2, SPMD across 8 NeuronCores.

out[b, h, i, j] = scores[b, h, i, j] - slope[h] * (i - j)

(The `offset` input cancels: (i+off) - (j+off) == i - j exactly in f32 for
integer offsets well inside the f32 exact-integer range.)

Sharding: flatten [B, H] = [2, 16] -> 32 slices; each of the 8 cores owns 4
consecutive (b, h) slices. The bias only depends on (h, i - j), so each core
receives a host-precomputed "strip" per slice: strip[p, c] = slope_h * (p - c
+ 1920) of shape [128, 3968]. For the row-tile starting at row r0, the bias
tile [128, 2048] is exactly strip[:, 1920-r0 : 1920-r0+2048], so the whole
kernel is one tensor_sub per tile.
"""

import numpy as np

_B, _H, _S = 2, 16, 2048
_NC = 8
_SPC = (_B * _H) // _NC  # slices (b,h pairs) per core = 4
_P = 128                 # SBUF partitions / row-tile height
_PAD = _S - _P           # 1920
_SW = _S + _PAD          # strip width 3968
_NRT = _S // _P          # row tiles per slice = 16

_CACHE = {}


def _build_nc():
    import concourse.tile as tile
    from concourse import bacc, mybir

    f32 = mybir.dt.float32
    nc = bacc.Bacc("TRN2", target_bir_lowering=False, debug=False)
    scores_in = nc.declare_dram_parameter("scores", [_SPC, _S, _S], f32, isOutput=False)
    slopes_in = nc.declare_dram_parameter("slopes", [_P, _SPC], f32, isOutput=False)
    out_ext = nc.declare_dram_parameter("out", [_SPC, _S, _S], f32, isOutput=True)

    with tile.TileContext(nc) as tc:
        with (
            tc.tile_pool(name="setup", bufs=1) as sup,
            tc.tile_pool(name="strip", bufs=1) as sp,
            tc.tile_pool(name="inp", bufs=6) as ip,
            tc.tile_pool(name="outp", bufs=6) as op,
        ):
            # base[p, c] = p - c + PAD, exact small integers in f32
            base = sup.tile([_P, _SW], f32)
            nc.gpsimd.iota(
                base[:],
                pattern=[[-1, _SW]],
                base=_PAD,
                channel_multiplier=1,
                allow_small_or_imprecise_dtypes=True,
            )
            slopes = sup.tile([_P, _SPC], f32)
            nc.sync.dma_start(slopes[:], slopes_in[:])
            # strip slice for local head hl: slope_hl * base
            strips = sp.tile([_P, _SPC * _SW], f32)
            for hl in range(_SPC):
                nc.vector.tensor_scalar_mul(
                    strips[:, hl * _SW : (hl + 1) * _SW],
                    base[:],
                    slopes[:, hl : hl + 1],
                )
            for hl in range(_SPC):
                for r in range(_NRT):
                    r0 = r * _P
                    t = ip.tile([_P, _S], f32)
                    nc.sync.dma_start(t[:], scores_in[hl, r0 : r0 + _P, :])
                    o = op.tile([_P, _S], f32)
                    off = hl * _SW + (_PAD - r0)
                    nc.vector.tensor_sub(o[:], t[:], strips[:, off : off + _S])
                    nc.sync.dma_start(out_ext[hl, r0 : r0 + _P, :], o[:])
    nc.compile()
    return nc


def _slopes_np():
    # slopes as the reference computes them (f32 throughout)
    slopes = (
        2.0 ** (-8.0 * np.arange(1, _H + 1, dtype=np.float32) / np.float32(_H))
    ).astype(np.float32)
    per_core = np.empty((_NC, _P, _SPC), dtype=np.float32)
    for core in range(_NC):
        for hl in range(_SPC):
            h = (core * _SPC + hl) % _H
            per_core[core, :, hl] = slopes[h]
    return per_core


def run(scores, offset=0, trace=False, trace_kwargs=None):
    """Run the SPMD kernel; returns (full_output, BassKernelResults)."""
    from concourse.bass_utils import run_bass_kernel_spmd

    scores = np.asarray(scores)
    assert scores.shape == (_B, _H, _S, _S) and scores.dtype == np.float32

    if "nc" not in _CACHE:
        _CACHE["nc"] = _build_nc()
        _CACHE["slopes"] = _slopes_np()
    nc = _CACHE["nc"]
    slopes = _CACHE["slopes"]

    flat = scores.reshape(_B * _H, _S, _S)
    in_maps = [
        {"scores": flat[c * _SPC : (c + 1) * _SPC], "slopes": slopes[c]}
        for c in range(_NC)
    ]
    res = run_bass_kernel_spmd(
        nc,
        in_maps,
        core_ids=list(range(_NC)),
        trace=trace,
        **(trace_kwargs or {}),
    )
    out = np.empty((_B * _H, _S, _S), dtype=np.float32)
    for c in range(_NC):
        out[c * _SPC : (c + 1) * _SPC] = res.results[c]["out"]
    return out.reshape(_B, _H, _S, _S), res


def kernel(scores, offset=0):
    out, _ = run(scores, offset=offset, trace=False)
    return out
